# revision 2
# baseline (speedup 1.0000x reference)
"""Confusion-matrix (joint histogram) kernel for Trainium2.

Math: out[b, i, j] = #{pixels p in batch b : yp[b,p] == i and y[b,p] == j}
for i, j in [0, 21). Inputs yp, y are [8, 2048, 2048] int32, values in [0, 21).

Per NeuronCore (core c processes batch c):
  - DMA int32 pixel chunks into SBUF; ScalarE converts to bf16,
  - feature planes in matmul-ready interleaved layout
    (planes[p, blk*BW + u*6 + g]), split across engines by feature kind:
      u < D:  delta planes  (x == u)        via DVE tensor_scalar(is_equal), 4x
      u >= D: sign-step planes sign(x-u+.5) via ScalarE activation(Sign), +-1
    The 21 features {delta_0..delta_{D-1}, sigma_D..sigma_20} span the
    one-hot basis, so host-side inversion recovers exact counts.
  - joint counts via TensorE: planes(yp)^T @ planes(y), 6 pixel-columns per
    matmul ([128, BW] x [128, 126]) accumulated into one PSUM [128, 126]
    f32 tile (exact integers < 2^24),
  - host: sum 6 diagonal 21x21 blocks (g::6), subtract sentinel-pad
    contribution from the sigma-sigma corner, invert the basis transform.
"""

import numpy as np

C = 21                  # classes
G = 6                   # pixel-column groups per matmul (G*C = 126 <= 128)
P = 128                 # partitions
BW = 128                # block width: 126 used + 2 pad (128 => FWL weight load)
FP = 756                # pixel-columns per tensor per chunk (divisible by 6)
N_ACT = 4               # top classes as sign-step planes on ScalarE
SENTINEL = 64           # int32 value outside [0, 21)

_CACHE = {}


def _build(
    n_free,
    fp=FP,
    bw=BW,
    n_act=N_ACT,
    work_cols=None,
    repeat=1,
    skip_mm=False,
):
    import concourse.bacc as bacc
    import concourse.mybir as mybir
    import concourse.tile as tile
    from contextlib import nullcontext

    if work_cols is None:
        work_cols = n_free

    nc = bacc.Bacc(
        "TRN2",
        target_bir_lowering=False,
        debug=False,
        enable_asserts=False,
        num_devices=8,
    )
    yp = nc.dram_tensor("yp", [P, n_free], mybir.dt.int32, kind="ExternalInput").ap()
    y = nc.dram_tensor("y", [P, n_free], mybir.dt.int32, kind="ExternalInput").ap()
    out = nc.dram_tensor("out", [P, 126], mybir.dt.float32, kind="ExternalOutput").ap()

    n_main = (work_cols // fp) * fp
    tail_cols = work_cols - n_main                   # < fp
    tail_pad = -tail_cols % G
    tail_w = tail_cols + tail_pad
    total_mms = (n_main // G) + (tail_w // G)

    bf16 = mybir.dt.bfloat16
    f32 = mybir.dt.float32
    i32 = mybir.dt.int32
    Copy = mybir.ActivationFunctionType.Copy
    Sign = mybir.ActivationFunctionType.Sign
    n_dve = C - n_act

    with tile.TileContext(nc) as tc:
        with (
            tc.tile_pool(name="psum", bufs=1, space="PSUM") as psum_pool,
            tc.tile_pool(name="cat", bufs=3) as cat_pool,
            tc.tile_pool(name="planes", bufs=2) as plane_pool,
            tc.tile_pool(name="singles", bufs=1) as singles,
        ):
            acc = psum_pool.tile([P, 126], f32)
            mm = 0
            rep_ctx = tc.For_i(0, repeat, 1) if repeat > 1 else nullcontext()

            with rep_ctx:

                def do_plane_chunk(cat32, w):
                    """cat32: [128, 2*w] int32 = [yp vals | y vals], w % 6 == 0.

                    planes[p, blk*BW + u*6 + g] = feat_u(vals[p, blk*6+g]),
                    blk in [0, 2*w/6). A-side = blks [0, w/6), B-side = rest.
                    Each matmul reads a contiguous [128, BW] slice.
                    """
                    nonlocal mm
                    nblk = 2 * w // G
                    cat16 = cat_pool.tile([P, 2 * fp], bf16, tag="cat16")
                    c16 = cat16[:, : 2 * w]
                    nc.scalar.activation(c16[:], cat32[:], Copy)
                    planes = plane_pool.tile([P, nblk_max * BW], bf16, tag="planes")
                    pl3 = planes[:, : nblk * BW].rearrange("p (b f) -> p b f", f=BW)
                    cat3 = c16[:].rearrange("p (b f) -> p b f", f=G)
                    if bw == 128:
                        # zero the 2 pad columns so FWL loads clean weights
                        nc.vector.memset(pl3[:, :, 126:128], 0.0)
                    for u in range(n_dve):
                        nc.vector.tensor_scalar(
                            pl3[:, :, u * G : (u + 1) * G],
                            cat3[:],
                            float(u),
                            None,
                            mybir.AluOpType.is_equal,
                        )
                    for u in range(n_dve, C):
                        nc.scalar.activation(
                            pl3[:, :, u * G : (u + 1) * G],
                            cat3[:],
                            Sign,
                            bias=0.5 - u,
                        )
                    half = w // G
                    for t in (range(0) if skip_mm else range(w // G)):
                        nc.tensor.matmul(
                            acc[:, :],
                            pl3[:, t, :bw].rearrange("p f -> p f"),
                            pl3[:, half + t, :126].rearrange("p f -> p f"),
                            start=(mm == 0),
                            stop=(mm == total_mms - 1),
                        )
                        mm += 1

                nblk_max = 2 * fp // G

                off = 0
                while off < n_main:
                    cat32 = cat_pool.tile([P, 2 * fp], i32, tag="cat32")
                    nc.sync.dma_start(cat32[:, :fp], yp[:, off : off + fp])
                    nc.sync.dma_start(cat32[:, fp:], y[:, off : off + fp])
                    do_plane_chunk(cat32, fp)
                    off += fp

                if tail_cols:
                    ct = cat_pool.tile([P, 2 * fp], i32, tag="cat32")
                    ctw = ct[:, : 2 * tail_w]
                    if tail_pad:
                        nc.vector.memset(ctw[:], SENTINEL)
                    nc.sync.dma_start(
                        ctw[:, :tail_cols], yp[:, n_main : n_main + tail_cols]
                    )
                    nc.sync.dma_start(
                        ctw[:, tail_w : tail_w + tail_cols],
                        y[:, n_main : n_main + tail_cols],
                    )
                    do_plane_chunk(ctw, tail_w)

            assert skip_mm or mm == total_mms
            res = singles.tile([P, 126], f32)
            if skip_mm:
                nc.vector.memset(res[:], 0.0)
            else:
                nc.vector.tensor_copy(res[:], acc[:, :])
            nc.sync.dma_start(out, res[:])

    nc.compile()
    return nc


def _basis_matrix(n_act):
    """Phi[u, x] = feature u evaluated at class value x."""
    n_dve = C - n_act
    phi = np.zeros((C, C), dtype=np.float64)
    for u in range(n_dve):
        phi[u, u] = 1.0
    for u in range(n_dve, C):
        for x in range(C):
            phi[u, x] = 1.0 if x >= u else -1.0
    return phi


def _get(n_free):
    if n_free not in _CACHE:
        _CACHE[n_free] = _build(n_free)
    return _CACHE[n_free]


def kernel(yp, y, res, n_classes, _trace=False):
    from concourse import bass_utils

    yp = np.ascontiguousarray(np.asarray(yp))
    y = np.ascontiguousarray(np.asarray(y))
    B = yp.shape[0]
    n_free = yp[0].size // P
    nc = _get(n_free)
    in_maps = [
        {"yp": yp[b].reshape(P, n_free), "y": y[b].reshape(P, n_free)}
        for b in range(B)
    ]
    r = bass_utils.run_bass_kernel_spmd(
        nc, in_maps, core_ids=list(range(B)), trace=_trace
    )

    # Host-side reconstruction
    n_act = N_ACT
    n_dve = C - n_act
    tail_cols = n_free % FP
    tail_pad = -tail_cols % G
    n_pad_px = P * tail_pad
    phi = _basis_matrix(n_act)
    phi_inv = np.linalg.inv(phi)

    outs = []
    for b in range(B):
        Pm = r.results[b]["out"].astype(np.float64)
        M = np.zeros((C, C), np.float64)
        for g in range(G):
            M += Pm[g:126:G, g:126:G]
        if n_pad_px and n_act:
            M[n_dve:, n_dve:] -= n_pad_px  # sentinel pad pairs hit sigma x sigma
        Cb = phi_inv @ M @ phi_inv.T
        outs.append(Cb)
    res_np = np.stack(outs).astype(np.float32)
    if _trace:
        kernel._last_results = r
    return res_np


# revision 4
# speedup vs baseline: 1.1543x; 1.1543x over previous
"""Confusion-matrix (joint histogram) kernel for Trainium2.

Math: out[b, i, j] = #{pixels p in batch b : yp[b,p] == i and y[b,p] == j}
for i, j in [0, 21). Inputs yp, y are [8, 2048, 2048] int32, values in [0, 21).

Per NeuronCore (core c processes batch c):
  - DMA int32 pixel chunks into SBUF; ScalarE converts to bf16,
  - feature planes in matmul-ready interleaved layout
    (planes[p, blk*BW + u*6 + g]), split across engines by feature kind:
      u < D:  delta planes  (x == u)        via DVE tensor_scalar(is_equal), 4x
      u >= D: sign-step planes sign(x-u+.5) via ScalarE activation(Sign), +-1
    The 21 features {delta_0..delta_{D-1}, sigma_D..sigma_20} span the
    one-hot basis, so host-side inversion recovers exact counts.
  - joint counts via TensorE: planes(yp)^T @ planes(y), 6 pixel-columns per
    matmul ([128, BW] x [128, 126]) accumulated into one PSUM [128, 126]
    f32 tile (exact integers < 2^24),
  - host: sum 6 diagonal 21x21 blocks (g::6), subtract sentinel-pad
    contribution from the sigma-sigma corner, invert the basis transform.
"""

import numpy as np

C = 21                  # classes
G = 6                   # pixel-column groups per matmul (G*C = 126 <= 128)
P = 128                 # partitions
BW = 128                # block width: 126 used + 2 pad (128 => FWL weight load)
FP = 756                # pixel-columns per tensor per chunk (divisible by 6)
N_ACT = 4               # top classes as sign-step planes on ScalarE
SENTINEL = 64           # int32 value outside [0, 21)

_CACHE = {}


def _build(
    n_free,
    fp=FP,
    bw=BW,
    n_act=N_ACT,
    work_cols=None,
    repeat=1,
    skip_mm=False,
):
    import concourse.bacc as bacc
    import concourse.mybir as mybir
    import concourse.tile as tile
    from contextlib import nullcontext

    if work_cols is None:
        work_cols = n_free

    nc = bacc.Bacc(
        "TRN2",
        target_bir_lowering=False,
        debug=False,
        enable_asserts=False,
        num_devices=8,
    )
    yp = nc.dram_tensor("yp", [P, n_free], mybir.dt.int32, kind="ExternalInput").ap()
    y = nc.dram_tensor("y", [P, n_free], mybir.dt.int32, kind="ExternalInput").ap()
    out = nc.dram_tensor("out", [P, 126], mybir.dt.float32, kind="ExternalOutput").ap()

    n_main = (work_cols // fp) * fp
    tail_cols = work_cols - n_main                   # < fp
    tail_pad = -tail_cols % G
    tail_w = tail_cols + tail_pad
    total_mms = (n_main // G) + (tail_w // G)

    bf16 = mybir.dt.bfloat16
    f32 = mybir.dt.float32
    i32 = mybir.dt.int32
    Copy = mybir.ActivationFunctionType.Copy
    Sign = mybir.ActivationFunctionType.Sign
    n_dve = C - n_act

    with tile.TileContext(nc) as tc:
        with (
            tc.tile_pool(name="psum", bufs=1, space="PSUM") as psum_pool,
            tc.tile_pool(name="cat", bufs=3) as cat_pool,
            tc.tile_pool(name="planes", bufs=2) as plane_pool,
            tc.tile_pool(name="singles", bufs=1) as singles,
        ):
            acc = psum_pool.tile([P, 126], f32)
            if n_act:
                bias_t = singles.tile([P, n_act], f32)
                for k in range(n_act):
                    nc.vector.memset(bias_t[:, k : k + 1], 0.5 - (C - n_act + k))
            mm = 0
            rep_ctx = tc.For_i(0, repeat, 1) if repeat > 1 else nullcontext()

            with rep_ctx:

                def do_plane_chunk(cat32, w):
                    """cat32: [128, 2*w] int32 = [yp vals | y vals], w % 6 == 0.

                    planes[p, blk*BW + u*6 + g] = feat_u(vals[p, blk*6+g]),
                    blk in [0, 2*w/6). A-side = blks [0, w/6), B-side = rest.
                    Each matmul reads a contiguous [128, BW] slice.
                    """
                    nonlocal mm
                    nblk = 2 * w // G
                    cat16 = cat_pool.tile([P, 2 * fp], bf16, tag="cat16")
                    c16 = cat16[:, : 2 * w]
                    nc.scalar.activation(c16[:], cat32[:], Copy)
                    planes = plane_pool.tile([P, nblk_max * BW], bf16, tag="planes")
                    pl3 = planes[:, : nblk * BW].rearrange("p (b f) -> p b f", f=BW)
                    cat3 = c16[:].rearrange("p (b f) -> p b f", f=G)
                    if bw == 128:
                        # zero the 2 pad columns so FWL loads clean weights
                        nc.vector.memset(pl3[:, :, 126:128], 0.0)
                    for u in range(n_dve):
                        nc.vector.tensor_scalar(
                            pl3[:, :, u * G : (u + 1) * G],
                            cat3[:],
                            float(u),
                            None,
                            mybir.AluOpType.is_equal,
                        )
                    for u in range(n_dve, C):
                        k = u - n_dve
                        nc.scalar.activation(
                            pl3[:, :, u * G : (u + 1) * G],
                            cat3[:],
                            Sign,
                            bias=bias_t[:, k : k + 1],
                        )
                    half = w // G
                    for t in (range(0) if skip_mm else range(w // G)):
                        nc.tensor.matmul(
                            acc[:, :],
                            pl3[:, t, :bw].rearrange("p f -> p f"),
                            pl3[:, half + t, :126].rearrange("p f -> p f"),
                            start=(mm == 0),
                            stop=(mm == total_mms - 1),
                        )
                        mm += 1

                nblk_max = 2 * fp // G

                off = 0
                while off < n_main:
                    cat32 = cat_pool.tile([P, 2 * fp], i32, tag="cat32")
                    nc.sync.dma_start(cat32[:, :fp], yp[:, off : off + fp])
                    nc.sync.dma_start(cat32[:, fp:], y[:, off : off + fp])
                    do_plane_chunk(cat32, fp)
                    off += fp

                if tail_cols:
                    ct = cat_pool.tile([P, 2 * fp], i32, tag="cat32")
                    ctw = ct[:, : 2 * tail_w]
                    if tail_pad:
                        nc.vector.memset(ctw[:], SENTINEL)
                    nc.sync.dma_start(
                        ctw[:, :tail_cols], yp[:, n_main : n_main + tail_cols]
                    )
                    nc.sync.dma_start(
                        ctw[:, tail_w : tail_w + tail_cols],
                        y[:, n_main : n_main + tail_cols],
                    )
                    do_plane_chunk(ctw, tail_w)

            assert skip_mm or mm == total_mms
            res = singles.tile([P, 126], f32)
            if skip_mm:
                nc.vector.memset(res[:], 0.0)
            else:
                nc.vector.tensor_copy(res[:], acc[:, :])
            nc.sync.dma_start(out, res[:])

    nc.compile()
    return nc


def _basis_matrix(n_act):
    """Phi[u, x] = feature u evaluated at class value x."""
    n_dve = C - n_act
    phi = np.zeros((C, C), dtype=np.float64)
    for u in range(n_dve):
        phi[u, u] = 1.0
    for u in range(n_dve, C):
        for x in range(C):
            phi[u, x] = 1.0 if x >= u else -1.0
    return phi


def _get(n_free):
    if n_free not in _CACHE:
        _CACHE[n_free] = _build(n_free)
    return _CACHE[n_free]


def kernel(yp, y, res, n_classes, _trace=False):
    from concourse import bass_utils

    yp = np.ascontiguousarray(np.asarray(yp))
    y = np.ascontiguousarray(np.asarray(y))
    B = yp.shape[0]
    n_free = yp[0].size // P
    nc = _get(n_free)
    in_maps = [
        {"yp": yp[b].reshape(P, n_free), "y": y[b].reshape(P, n_free)}
        for b in range(B)
    ]
    r = bass_utils.run_bass_kernel_spmd(
        nc, in_maps, core_ids=list(range(B)), trace=_trace
    )

    # Host-side reconstruction
    n_act = N_ACT
    n_dve = C - n_act
    tail_cols = n_free % FP
    tail_pad = -tail_cols % G
    n_pad_px = P * tail_pad
    phi = _basis_matrix(n_act)
    phi_inv = np.linalg.inv(phi)

    outs = []
    for b in range(B):
        Pm = r.results[b]["out"].astype(np.float64)
        M = np.zeros((C, C), np.float64)
        for g in range(G):
            M += Pm[g:126:G, g:126:G]
        if n_pad_px and n_act:
            M[n_dve:, n_dve:] -= n_pad_px  # sentinel pad pairs hit sigma x sigma
        Cb = phi_inv @ M @ phi_inv.T
        outs.append(Cb)
    res_np = np.stack(outs).astype(np.float32)
    if _trace:
        kernel._last_results = r
    return res_np


# revision 11
# speedup vs baseline: 1.2631x; 1.0943x over previous
"""Confusion-matrix (joint histogram) kernel for Trainium2.

Math: out[b, i, j] = #{pixels p in batch b : yp[b,p] == i and y[b,p] == j}
for i, j in [0, 21). Inputs yp, y are [8, 2048, 2048] int32, values in [0, 21).

Per NeuronCore (core c processes batch c):
  - DMA int32 pixel chunks into SBUF; ScalarE converts to bf16,
  - feature planes in matmul-ready interleaved layout
    (planes[p, blk*BW + u*6 + g]), split across engines by feature kind:
      u < D:  delta planes  (x == u)        via DVE tensor_scalar(is_equal), 4x
      u == D: constant ones plane           via one-time memset (pool bufs
              are reused round-robin, so the ones/pad columns persist)
      u > D:  sign-step planes sign(x-u+.5) via ScalarE activation(Sign), +-1
    The 21 features {delta_0..delta_{D-1}, 1, sigma_{D+1}..sigma_20} span the
    one-hot basis, so host-side inversion recovers exact counts.
  - joint counts via TensorE: planes(yp)^T @ planes(y), 6 pixel-columns per
    matmul ([128, BW] x [128, 126]) accumulated into one PSUM [128, 126]
    f32 tile (exact integers < 2^24),
  - host: sum 6 diagonal 21x21 blocks (g::6), subtract sentinel-pad
    contribution from the sigma-sigma corner, invert the basis transform.
"""

import numpy as np

C = 21                  # classes
G = 6                   # pixel-column groups per matmul (G*C = 126 <= 128)
P = 128                 # partitions
BW = 128                # block width: 126 used + 2 pad (128 => FWL weight load)
FP = 756                # pixel-columns per tensor per chunk (divisible by 6)
N_ACT = 4               # top classes as sign-step planes on ScalarE
SENTINEL = 64           # int32 value outside [0, 21)

_CACHE = {}


def _build(
    n_free,
    fp=FP,
    bw=BW,
    n_act=N_ACT,
    work_cols=None,
    repeat=1,
    skip_mm=False,
):
    import concourse.bacc as bacc
    import concourse.mybir as mybir
    import concourse.tile as tile
    from contextlib import nullcontext

    if work_cols is None:
        work_cols = n_free

    nc = bacc.Bacc(
        "TRN2",
        target_bir_lowering=False,
        debug=False,
        enable_asserts=False,
        num_devices=8,
    )
    yp = nc.dram_tensor("yp", [P, n_free], mybir.dt.int32, kind="ExternalInput").ap()
    y = nc.dram_tensor("y", [P, n_free], mybir.dt.int32, kind="ExternalInput").ap()
    out = nc.dram_tensor("out", [P, 126], mybir.dt.float32, kind="ExternalOutput").ap()

    n_main = (work_cols // fp) * fp
    tail_cols = work_cols - n_main                   # < fp
    tail_pad = -tail_cols % G
    tail_w = tail_cols + tail_pad
    total_mms = (n_main // G) + (tail_w // G)

    bf16 = mybir.dt.bfloat16
    f32 = mybir.dt.float32
    i32 = mybir.dt.int32
    Copy = mybir.ActivationFunctionType.Copy
    Sign = mybir.ActivationFunctionType.Sign
    n_dve = C - 1 - n_act        # ones plane sits at u = n_dve

    with tile.TileContext(nc) as tc:
        with (
            tc.tile_pool(name="psum", bufs=1, space="PSUM") as psum_pool,
            tc.tile_pool(name="cat", bufs=3) as cat_pool,
            tc.tile_pool(name="planes", bufs=2) as plane_pool,
            tc.tile_pool(name="singles", bufs=1) as singles,
        ):
            acc = psum_pool.tile([P, 126], f32)
            if n_act:
                bias_t = singles.tile([P, n_act], f32)
                for k in range(n_act):
                    nc.vector.memset(bias_t[:, k : k + 1], 0.5 - (n_dve + 1 + k))
            mm = 0
            chunk_idx = 0
            rep_ctx = tc.For_i(0, repeat, 1) if repeat > 1 else nullcontext()

            with rep_ctx:

                def do_plane_chunk(cat32, w):
                    """cat32: [128, 2*w] int32 = [yp vals | y vals], w % 6 == 0.

                    planes[p, blk*BW + u*6 + g] = feat_u(vals[p, blk*6+g]),
                    blk in [0, 2*w/6). A-side = blks [0, w/6), B-side = rest.
                    Each matmul reads a contiguous [128, BW] slice.
                    """
                    nonlocal mm, chunk_idx
                    nblk = 2 * w // G
                    cat16 = cat_pool.tile([P, 2 * fp], bf16, tag="cat16")
                    c16 = cat16[:, : 2 * w]
                    nc.scalar.activation(c16[:], cat32[:], Copy)
                    planes = plane_pool.tile([P, nblk_max * BW], bf16, tag="planes")
                    pl3 = planes[:, : nblk * BW].rearrange("p (b f) -> p b f", f=BW)
                    cat3 = c16[:].rearrange("p (b f) -> p b f", f=G)
                    if chunk_idx < 2:
                        # constant columns, written once per pool buffer: the
                        # ones plane (u = n_dve) and, for bw=128, the 2 pad
                        # columns. Pool bufs rotate round-robin so these
                        # persist; later chunks never touch them. Memset the
                        # full-size view so the tail chunk is covered too.
                        plf = planes[:, :].rearrange("p (b f) -> p b f", f=BW)
                        nc.vector.memset(
                            plf[:, :, n_dve * G : (n_dve + 1) * G], 1.0
                        )
                        if bw == 128:
                            nc.vector.memset(plf[:, :, 126:128], 0.0)
                    chunk_idx += 1
                    for u in range(n_dve):
                        nc.vector.tensor_scalar(
                            pl3[:, :, u * G : (u + 1) * G],
                            cat3[:],
                            float(u),
                            None,
                            mybir.AluOpType.is_equal,
                        )
                    for u in range(n_dve + 1, C):
                        k = u - n_dve - 1
                        nc.scalar.activation(
                            pl3[:, :, u * G : (u + 1) * G],
                            cat3[:],
                            Sign,
                            bias=bias_t[:, k : k + 1],
                        )
                    half = w // G
                    for t in (range(0) if skip_mm else range(w // G)):
                        nc.tensor.matmul(
                            acc[:, :],
                            pl3[:, t, :bw].rearrange("p f -> p f"),
                            pl3[:, half + t, :126].rearrange("p f -> p f"),
                            start=(mm == 0),
                            stop=(mm == total_mms - 1),
                        )
                        mm += 1

                nblk_max = 2 * fp // G

                off = 0
                while off < n_main:
                    cat32 = cat_pool.tile([P, 2 * fp], i32, tag="cat32")
                    nc.sync.dma_start(cat32[:, :fp], yp[:, off : off + fp])
                    nc.sync.dma_start(cat32[:, fp:], y[:, off : off + fp])
                    do_plane_chunk(cat32, fp)
                    off += fp

                if tail_cols:
                    ct = cat_pool.tile([P, 2 * fp], i32, tag="cat32")
                    ctw = ct[:, : 2 * tail_w]
                    if tail_pad:
                        nc.vector.memset(ctw[:], SENTINEL)
                    nc.sync.dma_start(
                        ctw[:, :tail_cols], yp[:, n_main : n_main + tail_cols]
                    )
                    nc.sync.dma_start(
                        ctw[:, tail_w : tail_w + tail_cols],
                        y[:, n_main : n_main + tail_cols],
                    )
                    do_plane_chunk(ctw, tail_w)

            assert skip_mm or mm == total_mms
            res = singles.tile([P, 126], f32)
            if skip_mm:
                nc.vector.memset(res[:], 0.0)
            else:
                nc.vector.tensor_copy(res[:], acc[:, :])
            nc.sync.dma_start(out, res[:])

    nc.compile()
    return nc


def _feature_eval(n_act, x):
    """Feature vector phi(x): deltas, ones, sign-steps. Works for any x
    (including the sentinel)."""
    n_dve = C - 1 - n_act
    v = np.zeros(C, dtype=np.float64)
    for u in range(n_dve):
        v[u] = 1.0 if x == u else 0.0
    v[n_dve] = 1.0
    for u in range(n_dve + 1, C):
        v[u] = 1.0 if x >= u else -1.0
    return v


def _basis_matrix(n_act):
    """Phi[u, x] = feature u evaluated at class value x."""
    return np.stack([_feature_eval(n_act, x) for x in range(C)], axis=1)


def _get(n_free):
    if n_free not in _CACHE:
        _CACHE[n_free] = _build(n_free)
    return _CACHE[n_free]


def kernel(yp, y, res, n_classes, _trace=False):
    from concourse import bass_utils

    yp = np.ascontiguousarray(np.asarray(yp))
    y = np.ascontiguousarray(np.asarray(y))
    B = yp.shape[0]
    n_free = yp[0].size // P
    nc = _get(n_free)
    in_maps = [
        {"yp": yp[b].reshape(P, n_free), "y": y[b].reshape(P, n_free)}
        for b in range(B)
    ]
    r = bass_utils.run_bass_kernel_spmd(
        nc, in_maps, core_ids=list(range(B)), trace=_trace
    )

    # Host-side reconstruction
    n_act = N_ACT
    tail_cols = n_free % FP
    tail_pad = -tail_cols % G
    n_pad_px = P * tail_pad
    phi = _basis_matrix(n_act)
    phi_inv = np.linalg.inv(phi)
    pad_vec = _feature_eval(n_act, SENTINEL)
    pad_corr = n_pad_px * np.outer(pad_vec, pad_vec)

    outs = []
    for b in range(B):
        Pm = r.results[b]["out"].astype(np.float64)
        M = np.zeros((C, C), np.float64)
        for g in range(G):
            M += Pm[g:126:G, g:126:G]
        M -= pad_corr
        Cb = phi_inv @ M @ phi_inv.T
        outs.append(Cb)
    res_np = np.stack(outs).astype(np.float32)
    if _trace:
        kernel._last_results = r
    return res_np
